# revision 42
# baseline (speedup 1.0000x reference)
"""CausalWanSelfAttention Trainium2 kernel — single SPMD launch on 8 NeuronCores.

Sharding: column-parallel QKV by heads. Each core owns 2 heads: one exclusive
"F" head plus one boundary "H" head shared with a sibling core; the H head's
output-projection weight is pre-scaled by 0.5 (and its RMSNorm sum-of-squares
contribution weighted 0.5) so summing the 8 partial outputs / statistics is
exact. RMSNorm statistics are combined with one tiny cross-core AllReduce.

Wire-byte minimization (the launch cost is dominated by the host<->device
tunnel: ~30-60 MB/s, plus a fixed per-array round-trip cost): x is shipped
as per-core fp16 [D, 512] shards of the 4096-padded transposed sequence plus
fp16 cos/sin half-row shards, assembled on device with one AllGather; Wq/Wk
ship int8 with per-column fp16 scales (dequantized on device; q/k
quantization error is softmax-damped) while Wv/Wo ship fp16; the projection
matmuls run natively in fp16 (fp32 PSUM accumulation). The per-core partial
output projection is ReduceScattered on device in fp16, then each core
quantizes its own [D, 512] L-shard to int8 with per-feature-row scales so
the D2H leg (and the donated zero-buffer upload) moves half the bytes; the
host dequantizes by the exported exact multipliers. All inputs are packed
into ONE uint8 blob per core and all outputs into one blob (device-side
bitcast views), so each wire leg pays a single per-array round-trip.

The block-sparse mask decomposes into 4 dense attention groups (no masking
inside a group), so softmax runs without max-subtraction (scores are O(1)
after RMSNorm; |s| <= sqrt(128)). Scores are computed in [kv, q] layout;
softmax denominators via a ones-matmul; per-query normalization is fused into
the PSUM->SBUF copy. Head dims are permuted (even dims then odd dims)
host-side so RoPE needs no strided ops. State tokens attend only to
themselves (softmax==1 -> o=v): handled on host from a tiny exported v_state.
Attention score/context matmuls run as float32r (full-rate fp32 mode).
"""
import sys
import numpy as np

sys.path.insert(0, "/opt/trn_rl_repo")

# ---- problem constants (hardcoded; kernel.py must be self-contained) ----
FS = 512
NIB = 3
NAPB = 32
L = 3683
LP = 3712           # 29 * 128 (compute padding)
LPAD = 4096         # 8 * 512  (wire/shard padding)
D = 1536
NH = 12
HD = 128
EPS = 1e-6
IB0 = FS                  # 512  image blocks start
A0 = FS + NIB * 2 * FS    # 3584 actions start
S0 = A0 + NIB * NAPB      # 3680 states start
NKT = D // 128            # 12 contraction tiles
NLT = LP // 128           # 29 L tiles
SCALE = float(1.0 / np.sqrt(HD))

CW2 = 256  # rope/normalize L-chunk width
SH = 512   # per-core L shard width (wire)
XR = D + 128  # rows per core in the AllGather buffer: xT shard + cos64 + sin64

# ---- single-blob wire layout (the axon tunnel charges ~67ms per ARRAY,
# so every tensor is packed into one uint8 input blob / one output blob) ----
_ISEGS = [  # name -> (bytes)
    ("xin", XR * SH * 2),        # f16 [1664, 512]
    ("wq8", D * 256),            # i8  [1536, 256]
    ("wk8", D * 256),            # i8  [1536, 256]
    ("wv", D * 256 * 2),         # f16 [1536, 256]
    ("wo", 128 * 3072 * 2),      # f16 [128, 3072]
    ("wqs", 512 * 2),            # f16 [1, 512]
    ("bqk", 128 * 4 * 4),        # f32 [128, 4]
    ("bv1", 256 * 4),            # f32 [1, 256]
    ("ones2", 128 * 2 * 4),      # f32 [128, 2]
]
IOFF = {}
_o = 0
for _n, _s in _ISEGS:
    IOFF[_n] = _o
    _o += _s
NB = _o                          # 4,068,352 bytes per core

OB_OUT8 = 0                      # i8  [1536, 512]
OB_OSCL = D * SH                 # f32 [128, 12]
OB_VST = OB_OSCL + 128 * NKT * 4  # f32 [3, 256]
OB = OB_VST + 3 * 256 * 4        # 795,648 bytes per core

# phase-1 chunks: one 512-wide chunk per AG block; block 7 only has 128
# valid columns (3584..3712) within the LP compute range.
CHUNKS = [(512 * ci, 512) for ci in range(7)] + [(3584, 128)]


def _mk_chunks(w):
    ch = [(i * w, w) for i in range(LP // w)] + [(LP - LP % w, LP % w)]
    return [(c, x) for (c, x) in ch if x > 0]

CHUNKS2 = _mk_chunks(CW2)

# core -> (F head, H head); H heads are computed on two cores each
CORE_HEADS = []
for _a in range(4):
    CORE_HEADS.append((3 * _a, 3 * _a + 1))
    CORE_HEADS.append((3 * _a + 2, 3 * _a + 1))


def _groups():
    """Dense attention groups: q ranges, kv 128-tile indices, runt kv info."""
    gs = [dict(q=[(0, 512)], kvt=list(range(4)), runt=None)]
    for b in range(NIB):
        be = IB0 + (b + 1) * 2 * FS
        kv0 = max(IB0, be - 4 * FS)
        if kv0 == IB0:
            tiles = list(range(be // 128))
        else:
            tiles = list(range(4)) + list(range(kv0 // 128, be // 128))
        q = [(IB0 + b * 2 * FS, 512), (IB0 + b * 2 * FS + 512, 512),
             (A0 + b * NAPB, NAPB)]
        gs.append(dict(q=q, kvt=tiles, runt=b))
    return gs

GROUPS = _groups()

_PROGRAM_CACHE = {}


def _install_jit_cache():
    """Cache the jitted shard_map callable across launches.

    bass2jax.run_bass_via_pjrt builds a fresh ``_body`` closure per call, so
    jax's C++ fast-path cache misses and every launch pays ~0.3s of retrace +
    executable-cache lookup (hashing the multi-MB BIR backend config).
    Reusing one jitted callable per (nc, n_cores) removes that; the per-call
    transfers and device execution are unchanged. Falls back to the original
    implementation for any case this specialized path doesn't cover.
    """
    if "patched" in _PROGRAM_CACHE:
        return
    _PROGRAM_CACHE["patched"] = True
    import jax
    from jax.sharding import Mesh, PartitionSpec
    try:
        from jax.shard_map import shard_map
    except Exception:
        from jax.experimental.shard_map import shard_map
    from concourse import bass2jax, mybir

    orig = bass2jax.run_bass_via_pjrt
    cache = {}

    def cached_run(nc, in_maps, n_cores):
        try:
            key = (id(nc), n_cores)
            ent = cache.get(key)
            if ent is None:
                if nc.dbg_addr is not None or n_cores <= 1:
                    raise RuntimeError("unsupported; use original")
                bass2jax.install_neuronx_cc_hook()
                pname = (nc.partition_id_tensor.name
                         if nc.partition_id_tensor else None)
                in_names, out_names, out_avals = [], [], []
                for alloc in nc.m.functions[0].allocations:
                    if not isinstance(alloc, mybir.MemoryLocationSet):
                        continue
                    name = alloc.memorylocations[0].name
                    if alloc.kind == "ExternalInput":
                        if name != pname:
                            in_names.append(name)
                    elif alloc.kind == "ExternalOutput":
                        out_names.append(name)
                        out_avals.append(jax.core.ShapedArray(
                            tuple(alloc.tensor_shape),
                            mybir.dt.np(alloc.dtype)))
                n_params = len(in_names)
                n_outs = len(out_avals)
                all_names = in_names + out_names + ([pname] if pname else [])
                import jax.numpy as jnp

                def _body(*args):
                    # the kernel writes every output byte, so the output
                    # operands can be zero-materialized ON DEVICE instead of
                    # uploading donated host zero buffers over the tunnel
                    operands = list(args)
                    operands += [jnp.zeros(a.shape, a.dtype)
                                 for a in out_avals]
                    if pname is not None:
                        operands.append(bass2jax.partition_id_tensor())
                    return tuple(bass2jax._bass_exec_p.bind(
                        *operands, out_avals=tuple(out_avals),
                        in_names=tuple(all_names),
                        out_names=tuple(out_names),
                        lowering_input_output_aliases=(),
                        sim_require_finite=True, sim_require_nnan=True,
                        nc=nc))

                devices = jax.devices()[:n_cores]
                assert len(devices) == n_cores
                mesh = Mesh(np.asarray(devices), ("core",))
                sharded = jax.jit(
                    shard_map(_body, mesh=mesh,
                              in_specs=(PartitionSpec("core"),) * n_params,
                              out_specs=(PartitionSpec("core"),) * n_outs,
                              check_rep=False),
                    keep_unused=True)
                ent = cache[key] = (sharded, in_names, out_names, out_avals)
            if ent == "disabled":
                return orig(nc, in_maps, n_cores=n_cores)
            sharded, in_names, out_names, out_avals = ent
            per_core = [[np.asarray(m[n]) for n in in_names] for m in in_maps]
            concat_in = [
                np.concatenate([per_core[c][i] for c in range(n_cores)], axis=0)
                for i in range(len(in_names))]
            try:
                out_arrs = sharded(*concat_in)
            except Exception:
                cache[key] = "disabled"
                raise
            return [
                {name: np.asarray(out_arrs[i]).reshape(
                    n_cores, *out_avals[i].shape)[c]
                 for i, name in enumerate(out_names)}
                for c in range(n_cores)]
        except Exception:
            return orig(nc, in_maps, n_cores=n_cores)

    bass2jax.run_bass_via_pjrt = cached_run


def _build_program():
    import concourse.bacc as bacc
    import concourse.tile as tile
    from concourse import mybir

    F16 = mybir.dt.float16
    F32 = mybir.dt.float32
    F32R = mybir.dt.float32r
    I8 = mybir.dt.int8
    AF = mybir.ActivationFunctionType

    nc = bacc.Bacc("TRN2", target_bir_lowering=False, debug=False, num_devices=8)

    U8 = mybir.dt.uint8

    # single packed wire input / output (one array each way: the tunnel
    # charges a fixed ~67ms round-trip per array regardless of size)
    blob = nc.dram_tensor("blob", [1, NB], U8, kind="ExternalInput")
    oblob = nc.dram_tensor("oblob", [1, OB], U8, kind="ExternalOutput")

    def iseg(name, nbytes, dt_, rearr=None, **kw):
        ap = blob.ap()[0:1, IOFF[name]:IOFF[name] + nbytes].bitcast(dt_)
        return ap.rearrange(rearr, **kw) if rearr else ap

    xin_v = iseg("xin", XR * SH * 2, F16, "a (r c) -> (a r) c", r=XR)
    wq8_v = iseg("wq8", D * 256, I8, "a (kt p c) -> (a p) kt c", kt=NKT, p=128)
    wk8_v = iseg("wk8", D * 256, I8, "a (kt p c) -> (a p) kt c", kt=NKT, p=128)
    wv_v = iseg("wv", D * 256 * 2, F16, "a (kt p c) -> (a p) kt c", kt=NKT, p=128)
    wo_v = iseg("wo", 128 * 3072 * 2, F16, "a (p c) -> (a p) c", p=128)
    wqs_v = iseg("wqs", 512 * 2, F16)
    bqk_v = iseg("bqk", 128 * 4 * 4, F32, "a (p c) -> (a p) c", p=128)
    bv1_v = iseg("bv1", 256 * 4, F32)
    ones2_v = iseg("ones2", 128 * 2 * 4, F32R, "a (p c) -> (a p) c", p=128)

    def oseg(off, nbytes, dt_, p):
        return oblob.ap()[0:1, off:off + nbytes].bitcast(dt_).rearrange(
            "a (p c) -> (a p) c", p=p)

    RG = [list(range(8))]

    with tile.TileContext(nc) as tc:
        with tc.tile_pool(name="persist", bufs=1) as P, \
             tc.tile_pool(name="xin_p", bufs=1) as XP, \
             tc.tile_pool(name="tmp", bufs=2) as T, \
             tc.tile_pool(name="pt", bufs=3) as PT, \
             tc.tile_pool(name="osb", bufs=2) as OSB, \
             tc.tile_pool(name="ps", bufs=2, space="PSUM") as PSY, \
             tc.tile_pool(name="dram", bufs=1, space="DRAM") as DR:

            # ---------- collective buffers ----------
            agin = DR.tile([XR, SH], F16, name="agin")
            agout = DR.tile([8 * XR, SH], F16, addr_space="Shared", name="agout")
            opart = DR.tile([8 * D, SH], F16, name="opart")
            rsout = DR.tile([D, SH], F16, name="rsout")
            cin = DR.tile([1, 2 * LP], F32, name="cin")
            cout = DR.tile([1, 2 * LP], F32, name="cout")

            # stage the wire shard into an internal tile, then AllGather
            nc.sync.dma_start(agin[:], xin_v)
            nc.gpsimd.collective_compute(
                "AllGather", mybir.AluOpType.bypass,
                replica_groups=RG, ins=[agin.opt()], outs=[agout.opt()])

            # ---------- phase-1-resident SBUF ----------
            wq16 = P.tile([128, NKT, 256], F16, tag="wq16", name="wq16")
            wk16 = P.tile([128, NKT, 256], F16, tag="wk16", name="wk16")
            wv16 = P.tile([128, NKT, 256], F16, tag="wv16", name="wv16")
            bqk_sb = P.tile([128, 4], F32, tag="bqk")
            bv1_sb = P.tile([1, 256], F32, tag="bv1")
            bv_sb = P.tile([128, 256], F32, tag="bv")
            ones2 = P.tile([128, 2], F32R, tag="ones2")
            zero16 = P.tile([128, SH - 96], F16, tag="zero16")
            # whole-kernel-resident
            y_q = [P.tile([128, LP], F32R, tag=f"yq{u}", name=f"yq{u}") for u in range(2)]
            y_k = [P.tile([128, LP], F32R, tag=f"yk{u}", name=f"yk{u}") for u in range(2)]
            v_sb = P.tile([128, NLT, 256], F32R, tag="vsb")

            # Wq/Wk arrive int8 with per-column fp16 scales; dequant to fp16
            # (error from q/k quantization is softmax-damped; Wv/Wo stay fp16)
            w8q_sb = P.tile([128, NKT, 256], I8, tag="w8q")
            w8k_sb = P.tile([128, NKT, 256], I8, tag="w8k")
            qks_sb = P.tile([1, 512], F16, tag="qks")
            qksb = P.tile([128, 512], F16, tag="qksb")
            nc.sync.dma_start(w8q_sb[:], wq8_v)
            nc.sync.dma_start(w8k_sb[:], wk8_v)
            nc.sync.dma_start(qks_sb[:], wqs_v)
            nc.gpsimd.partition_broadcast(qksb[:], qks_sb[:])
            for kt in range(NKT):
                for w8s, w16s, s0 in ((w8q_sb, wq16, 0), (w8k_sb, wk16, 256)):
                    wdq = T.tile([128, 256], F16, tag="wdq")
                    nc.vector.tensor_copy(wdq[:], w8s[:, kt, :])
                    nc.vector.tensor_mul(w16s[:, kt, :], wdq[:],
                                         qksb[:, s0:s0 + 256])
            nc.sync.dma_start(wv16[:], wv_v)
            nc.sync.dma_start(bqk_sb[:], bqk_v)
            nc.sync.dma_start(bv1_sb[:], bv1_v)
            nc.gpsimd.partition_broadcast(bv_sb[:], bv1_sb[:])
            nc.sync.dma_start(ones2[:], ones2_v)
            nc.vector.memset(zero16[:], 0.0)

            # zero the ReduceScatter input columns phase 3 never writes
            # (q in [3680, 4096) of block 7: states + wire padding)
            for m in range(NKT):
                r0 = 7 * D + m * 128
                nc.sync.dma_start(opart[r0:r0 + 128, 96:SH], zero16[:])

            # ---------- phase 1: projections + ssq partials ----------
            for ci, (c0, cw) in enumerate(CHUNKS):
                xr0 = XR * ci
                xc16 = XP.tile([128, NKT, SH], F16, tag="xc16")
                nc.sync.dma_start(
                    xc16[:, :, 0:cw],
                    agout[xr0:xr0 + D, 0:cw].rearrange("(kt p) l -> p kt l", p=128))
                for ti, (w16, ys) in enumerate([(wq16, y_q), (wk16, y_k)]):
                    ssq_ps = PSY.tile([1, 512], F32, tag="ssqps")
                    for u in range(2):
                        yp = PSY.tile([128, 512], F32, tag="yp")
                        for kt in range(NKT):
                            nc.tensor.matmul(
                                yp[:, 0:cw], w16[:, kt, u * 128:(u + 1) * 128],
                                xc16[:, kt, 0:cw],
                                start=(kt == 0), stop=(kt == NKT - 1))
                        nc.vector.tensor_scalar_add(
                            ys[u][:, c0:c0 + cw], yp[:, 0:cw],
                            bqk_sb[:, 2 * ti + u:2 * ti + u + 1])
                        y2 = T.tile([128, 512], F32R, tag="y2")
                        nc.scalar.activation(y2[:, 0:cw],
                                             ys[u][:, c0:c0 + cw].bitcast(F32),
                                             AF.Square)
                        nc.tensor.matmul(ssq_ps[:, 0:cw], ones2[:, u:u + 1],
                                         y2[:, 0:cw], start=(u == 0), stop=(u == 1),
                                         skip_group_check=True)
                    ssq_st = T.tile([1, 512], F32, tag="ssqst")
                    nc.vector.tensor_copy(ssq_st[:, 0:cw], ssq_ps[:, 0:cw])
                    nc.sync.dma_start(cin[0:1, ti * LP + c0:ti * LP + c0 + cw], ssq_st[:, 0:cw])
                for lt in range(c0 // 128, (c0 + cw) // 128):
                    vp = PSY.tile([128, 512], F32, tag="vp", name="vp")[:, 0:256]
                    loff = lt * 128 - c0
                    for kt in range(NKT):
                        nc.tensor.matmul(vp[:], xc16[:, kt, loff:loff + 128],
                                         wv16[:, kt, :],
                                         start=(kt == 0), stop=(kt == NKT - 1))
                    nc.vector.tensor_add(v_sb[:, lt, :], vp[:], bv_sb[:])

            nc.sync.dma_start(oseg(OB_VST, 3 * 256 * 4, F32, 3),
                              v_sb[96:99, 28, :].bitcast(F32))

            # ---------- collective: AllReduce the ssq partials ----------
            nc.gpsimd.collective_compute(
                "AllReduce", mybir.AluOpType.add,
                replica_groups=RG, ins=[cin.opt()], outs=[cout.opt()])
            eps_t = P.tile([1, 1], F32, tag="epst")
            nc.vector.memset(eps_t[:], float(EPS))

            # cos/sin fp16 shards ride in the AllGather buffer; expand the
            # duplicated halves and upconvert into SBUF (reuse weight slots)
            cos_sb = P.tile([128, LP], F32, tag="wq16", name="cos_sb")
            sin_sb = P.tile([128, LP], F32, tag="wk16", name="sin_sb")
            for ci, (c0, cw) in enumerate(CHUNKS):
                tr0 = XR * ci + D
                cs16 = T.tile([128, SH], F16, tag="cs16")
                nc.sync.dma_start(cs16[0:64, 0:cw], agout[tr0:tr0 + 64, 0:cw])
                nc.sync.dma_start(cs16[64:128, 0:cw], agout[tr0:tr0 + 64, 0:cw])
                nc.vector.tensor_copy(cos_sb[:, c0:c0 + cw], cs16[:, 0:cw])
                sn16 = T.tile([128, SH], F16, tag="sn16")
                nc.sync.dma_start(sn16[0:64, 0:cw], agout[tr0 + 64:tr0 + 128, 0:cw])
                nc.sync.dma_start(sn16[64:128, 0:cw], agout[tr0 + 64:tr0 + 128, 0:cw])
                nc.vector.tensor_copy(sin_sb[:, c0:c0 + cw], sn16[:, 0:cw])

            # ---------- phase 2: normalize + rope (in place on y) ----------
            for (c0, cw) in CHUNKS2:
                for ti, ys in enumerate([y_q, y_k]):
                    s1 = T.tile([1, CW2], F32, tag="s1")
                    nc.sync.dma_start(s1[:, 0:cw],
                                      cout[0:1, ti * LP + c0:ti * LP + c0 + cw])
                    nc.scalar.activation(s1[:, 0:cw], s1[:, 0:cw], AF.Sqrt,
                                         bias=eps_t[:, 0:1], scale=float(1.0 / D))
                    nc.vector.reciprocal(s1[:, 0:cw], s1[:, 0:cw])
                    fb = T.tile([128, CW2], F32, tag="fb")
                    nc.gpsimd.partition_broadcast(fb[:, 0:cw], s1[:, 0:cw])
                    for u in range(2):
                        y = ys[u]
                        nc.vector.tensor_mul(y[:, c0:c0 + cw],
                                             y[:, c0:c0 + cw].bitcast(F32),
                                             fb[:, 0:cw])
                        ta = T.tile([128, CW2], F32, tag="ropea")
                        tb = T.tile([128, CW2], F32, tag="ropeb")
                        tbs = T.tile([128, CW2], F32, tag="ropec")
                        yv = y[:, c0:c0 + cw].bitcast(F32)
                        nc.vector.tensor_mul(ta[:, 0:cw], yv, cos_sb[:, c0:c0 + cw])
                        nc.vector.tensor_mul(tb[:, 0:cw], yv, sin_sb[:, c0:c0 + cw])
                        nc.sync.dma_start(tbs[0:64, 0:cw], tb[64:128, 0:cw])
                        nc.sync.dma_start(tbs[64:128, 0:cw], tb[0:64, 0:cw])
                        nc.vector.tensor_sub(y[0:64, c0:c0 + cw],
                                             ta[0:64, 0:cw], tbs[0:64, 0:cw])
                        nc.vector.tensor_add(y[64:128, c0:c0 + cw],
                                             ta[64:128, 0:cw], tbs[64:128, 0:cw])

            # Wo stays fp16 (o-projection matmuls run in fp16); reuse wv slot
            wo16 = P.tile([128, 3072], F16, tag="wv16", name="wo16")
            nc.sync.dma_start(wo16[:], wo_v)

            # ---------- phase 3: attention + partial o-projection ----------
            for g in GROUPS:
                runts = []
                if g["runt"] is not None:
                    b = g["runt"]
                    a_lo = A0 + b * NAPB
                    s_row = S0 + b
                    for u in range(2):
                        kr = T.tile([128, 33], F32R, tag=f"kr{u}")
                        nc.vector.tensor_copy(kr[:, 0:32],
                                              y_k[u][:, a_lo:a_lo + 32].bitcast(F32))
                        nc.vector.tensor_copy(kr[:, 32:33],
                                              y_k[u][:, s_row:s_row + 1].bitcast(F32))
                        vr = T.tile([33, 256], F32R, tag=f"vr{u}")
                        # partition-shifting copies must go through DMA
                        nc.sync.dma_start(
                            vr[0:32, :], v_sb[32 * b:32 * b + 32, 28, :])
                        nc.sync.dma_start(
                            vr[32:33, :], v_sb[96 + b:97 + b, 28, :])
                        runts.append((kr, vr))

                kvts = g["kvt"] + ([None] if g["runt"] is not None else [])
                for (q0, qw) in g["q"]:
                    o_sb = []
                    for u in range(2):
                        oT_ps = PSY.tile([128, 512], F32, tag="vp", name="oT_ps")
                        sm_ps = PSY.tile([1, 512], F32, tag="ssqps", name="sm_ps")
                        for i, t in enumerate(kvts):
                            if t is None:
                                klhs = runts[u][0][:, :]
                                vlhs = runts[u][1][:, u * 128:(u + 1) * 128]
                                kvn = 33
                            else:
                                klhs = y_k[u][:, t * 128:(t + 1) * 128]
                                vlhs = v_sb[:, t, u * 128:(u + 1) * 128]
                                kvn = 128
                            s_ps = PSY.tile([128, 512], F32, tag="yp", name="s_ps")
                            nc.tensor.matmul(s_ps[0:kvn, 0:qw], klhs,
                                             y_q[u][:, q0:q0 + qw],
                                             start=True, stop=True)
                            pT = PT.tile([128, 512], F32R, tag="pT")
                            nc.scalar.activation(pT[0:kvn, 0:qw],
                                                 s_ps[0:kvn, 0:qw], AF.Exp,
                                                 scale=SCALE)
                            nc.tensor.matmul(oT_ps[:, 0:qw], vlhs, pT[0:kvn, 0:qw],
                                             start=(i == 0), stop=(i == len(kvts) - 1),
                                             skip_group_check=True)
                            nc.tensor.matmul(sm_ps[:, 0:qw], ones2[0:kvn, 0:1],
                                             pT[0:kvn, 0:qw],
                                             start=(i == 0), stop=(i == len(kvts) - 1),
                                             skip_group_check=True)
                        sm_sb = T.tile([1, 512], F32, tag="smsb")
                        nc.vector.reciprocal(sm_sb[:, 0:qw], sm_ps[:, 0:qw])
                        rb = T.tile([128, 512], F32, tag="rb")
                        nc.gpsimd.partition_broadcast(rb[:, 0:qw], sm_sb[:, 0:qw])
                        ot = OSB.tile([128, 512], F16, tag="ot")
                        nc.vector.tensor_mul(ot[:, 0:qw], oT_ps[:, 0:qw], rb[:, 0:qw])
                        o_sb.append(ot)
                    blk = q0 // SH
                    l0 = q0 - blk * SH
                    for m in range(NKT):
                        op_ps = PSY.tile([128, 512], F32, tag="op", name="op_ps")
                        for u in range(2):
                            nc.tensor.matmul(
                                op_ps[:, 0:qw],
                                wo16[:, u * D + m * 128:u * D + (m + 1) * 128],
                                o_sb[u][:, 0:qw],
                                start=(u == 0), stop=(u == 1))
                        op16 = OSB.tile([128, 512], F16, tag="opsb", name="op16")
                        nc.vector.tensor_copy(op16[:, 0:qw], op_ps[:, 0:qw])
                        r0 = blk * D + m * 128
                        nc.sync.dma_start(opart[r0:r0 + 128, l0:l0 + qw],
                                          op16[:, 0:qw])

            # ---------- collective: ReduceScatter the output partials ----------
            nc.gpsimd.collective_compute(
                "ReduceScatter", mybir.AluOpType.add,
                replica_groups=RG, ins=[opart.opt()], outs=[rsout.opt()])

            # quantize this core's output shard to int8 with per-feature-row
            # scales (wire compression for the D2H leg)
            ro16 = XP.tile([128, NKT, SH], F16, tag="xc16", name="ro16")
            nc.sync.dma_start(
                ro16[:], rsout[:].rearrange("(t p) l -> p t l", p=128))
            # oscl exports the exact multiplier used (host divides by it), so
            # the only round-trip error is the int8 rounding itself
            oscl_sb = P.tile([128, NKT], F32, tag="osclsb")
            for t in range(NKT):
                mx = T.tile([128, 1], F32, tag="mx")
                nc.vector.tensor_reduce(mx[:], ro16[:, t, :],
                                        axis=mybir.AxisListType.X,
                                        op=mybir.AluOpType.max,
                                        apply_absolute_value=True)
                nc.vector.tensor_scalar_max(mx[:], mx[:], 1e-2)
                rr = T.tile([128, 1], F32, tag="rr")
                nc.vector.reciprocal(rr[:], mx[:])
                nc.vector.tensor_scalar_mul(rr[:], rr[:], 127.0)
                nc.vector.tensor_copy(oscl_sb[:, t:t + 1], rr[:])
                q8t = OSB.tile([128, SH], I8, tag="q8t")
                nc.vector.tensor_scalar_mul(q8t[:], ro16[:, t, :], rr[:, 0:1])
                nc.sync.dma_start(oseg(128 * SH * t, 128 * SH, I8, 128), q8t[:])
            nc.sync.dma_start(oseg(OB_OSCL, 128 * NKT * 4, F32, 128),
                              oscl_sb[:])

    nc.finalize()
    return nc


def _prep_inputs(x, freqs, freqs_action, freqs_state, Wq, bq, Wk, bk, Wv, bv,
                 Wo, bo, gq, gk):
    """Host-side input prep -> per-core in_maps. gq/gk are ones (per spec)."""
    x = np.ascontiguousarray(np.asarray(x, np.float32)[0])
    xT16 = np.zeros((D, LPAD), np.float16)
    xT16[:, :L] = x.T.astype(np.float16)
    f = np.concatenate([np.asarray(freqs), np.asarray(freqs_action),
                        np.asarray(freqs_state)], 0).astype(np.float32)
    f = f.reshape(L, HD // 2, 2)
    cos64 = np.zeros((64, LPAD), np.float16)
    sin64 = np.zeros((64, LPAD), np.float16)
    cos64[:, :L] = f[..., 0].T.astype(np.float16)
    sin64[:, :L] = f[..., 1].T.astype(np.float16)
    perm = np.concatenate([np.arange(0, HD, 2), np.arange(1, HD, 2)])
    ones2 = np.ones((128, 2), np.float32)
    ones2[:, 1] = 0.5

    Wq = np.asarray(Wq, np.float32); Wk = np.asarray(Wk, np.float32)
    Wv = np.asarray(Wv, np.float32); Wo = np.asarray(Wo, np.float32)
    bq = np.asarray(bq, np.float32); bk = np.asarray(bk, np.float32)
    bv = np.asarray(bv, np.float32)

    def quant8(w):
        # per-column symmetric int8; scale kept in fp16 (as the device uses it)
        s = (np.abs(w).max(0) / 127.0).astype(np.float16)
        s32 = np.maximum(s.astype(np.float32), 1e-12)
        q = np.clip(np.round(w / s32[None, :]), -127, 127).astype(np.int8)
        return q, s

    in_maps = []
    for c in range(8):
        F, H = CORE_HEADS[c]
        pf = F * HD + perm
        ph = H * HD + perm
        vcols = np.r_[F * HD:(F + 1) * HD, H * HD:(H + 1) * HD]
        sl = slice(SH * c, SH * (c + 1))
        q8, qs = quant8(np.concatenate([Wq[:, pf], Wq[:, ph]], 1))
        k8, ks = quant8(np.concatenate([Wk[:, pf], Wk[:, ph]], 1))
        segs = {
            "xin": np.ascontiguousarray(np.concatenate(
                [xT16[:, sl], cos64[:, sl], sin64[:, sl]], 0)),
            "wq8": np.ascontiguousarray(q8),
            "wk8": np.ascontiguousarray(k8),
            "wv": np.ascontiguousarray(Wv[:, vcols]).astype(np.float16),
            "wo": np.ascontiguousarray(np.concatenate(
                [Wo[F * HD:(F + 1) * HD, :], 0.5 * Wo[H * HD:(H + 1) * HD, :]],
                1)).astype(np.float16),
            "wqs": np.ascontiguousarray(np.concatenate([qs, ks])[None, :]),
            "bqk": np.ascontiguousarray(
                np.stack([bq[pf], bq[ph], bk[pf], bk[ph]], 1).astype(np.float32)),
            "bv1": np.ascontiguousarray(bv[vcols][None, :].astype(np.float32)),
            "ones2": ones2,
        }
        blob = np.concatenate(
            [np.ascontiguousarray(segs[n]).view(np.uint8).reshape(1, -1)
             for n, _ in _ISEGS], axis=1)
        assert blob.shape == (1, NB)
        in_maps.append({"blob": blob})
    return in_maps


def kernel(**inputs) -> np.ndarray:
    from concourse.bass_utils import run_bass_kernel_spmd

    _install_jit_cache()
    if "nc" not in _PROGRAM_CACHE:
        _PROGRAM_CACHE["nc"] = _build_program()
    nc = _PROGRAM_CACHE["nc"]

    in_maps = _prep_inputs(**inputs)
    res = run_bass_kernel_spmd(nc, in_maps, core_ids=list(range(8)))

    Wo = np.asarray(inputs["Wo"], np.float32)
    bo = np.asarray(inputs["bo"], np.float32)
    outT = np.zeros((D, LPAD), np.float32)
    vsts = []
    for c in range(8):
        ob = res.results[c]["oblob"][0]
        q8 = ob[:OB_OSCL].view(np.int8).reshape(D, SH).astype(np.float32)
        rr = ob[OB_OSCL:OB_VST].view(np.float32).reshape(128, NKT)
        vsts.append(ob[OB_VST:].view(np.float32).reshape(3, 256))
        s = np.ascontiguousarray(rr.T).reshape(D, 1)   # feature d = 128*t + p
        outT[:, SH * c:SH * (c + 1)] = q8 / s
    out = np.zeros((L, D), np.float32)
    out[:S0] = outT[:, :S0].T
    v_state = np.zeros((3, D), np.float32)
    have = set()
    for c in range(8):
        F, H = CORE_HEADS[c]
        vs = vsts[c]
        if F not in have:
            v_state[:, F * HD:(F + 1) * HD] = vs[:, :HD]
            have.add(F)
        if H not in have:
            v_state[:, H * HD:(H + 1) * HD] = vs[:, HD:]
            have.add(H)
    out[S0:S0 + NIB] = v_state @ Wo
    out += bo[None, :]
    return out[None].astype(np.float32)


# revision 43
# speedup vs baseline: 1.3002x; 1.3002x over previous
"""CausalWanSelfAttention Trainium2 kernel — single SPMD launch on 8 NeuronCores.

Sharding: column-parallel QKV by heads. Each core owns 2 heads: one exclusive
"F" head plus one boundary "H" head shared with a sibling core; the H head's
output-projection weight is pre-scaled by 0.5 (and its RMSNorm sum-of-squares
contribution weighted 0.5) so summing the 8 partial outputs / statistics is
exact. RMSNorm statistics are combined with one tiny cross-core AllReduce.

Wire-byte minimization (the launch cost is dominated by the host<->device
tunnel: ~30-60 MB/s, plus a fixed per-array round-trip cost): x is shipped
as per-core fp16 [D, 512] shards of the 4096-padded transposed sequence plus
fp16 cos/sin half-row shards, assembled on device with one AllGather; Wq/Wk
ship int8 with per-column fp16 scales (dequantized on device; q/k
quantization error is softmax-damped) while Wv/Wo ship fp16; the projection
matmuls run natively in fp16 (fp32 PSUM accumulation). The per-core partial
output projection is ReduceScattered on device in fp16, then each core
quantizes its own [D, 512] L-shard to int8 with per-feature-row scales so
the D2H leg (and the donated zero-buffer upload) moves half the bytes; the
host dequantizes by the exported exact multipliers. All inputs are packed
into ONE uint8 blob per core and all outputs into one blob (device-side
bitcast views), so each wire leg pays a single per-array round-trip.

The block-sparse mask decomposes into 4 dense attention groups (no masking
inside a group), so softmax runs without max-subtraction (scores are O(1)
after RMSNorm; |s| <= sqrt(128)). Scores are computed in [kv, q] layout;
softmax denominators via a ones-matmul; per-query normalization is fused into
the PSUM->SBUF copy. Head dims are permuted (even dims then odd dims)
host-side so RoPE needs no strided ops. State tokens attend only to
themselves (softmax==1 -> o=v): handled on host from a tiny exported v_state.
Attention score/context matmuls run as float32r (full-rate fp32 mode).
"""
import sys
import numpy as np

sys.path.insert(0, "/opt/trn_rl_repo")

# ---- problem constants (hardcoded; kernel.py must be self-contained) ----
FS = 512
NIB = 3
NAPB = 32
L = 3683
LP = 3712           # 29 * 128 (compute padding)
LPAD = 4096         # 8 * 512  (wire/shard padding)
D = 1536
NH = 12
HD = 128
EPS = 1e-6
IB0 = FS                  # 512  image blocks start
A0 = FS + NIB * 2 * FS    # 3584 actions start
S0 = A0 + NIB * NAPB      # 3680 states start
NKT = D // 128            # 12 contraction tiles
NLT = LP // 128           # 29 L tiles
SCALE = float(1.0 / np.sqrt(HD))

CW2 = 256  # rope/normalize L-chunk width
SH = 512   # per-core L shard width (wire)
XR = D + 128  # rows per core in the AllGather buffer: xT shard + cos64 + sin64

# ---- single-blob wire layout (the axon tunnel charges ~67ms per ARRAY,
# so every tensor is packed into one uint8 input blob / one output blob) ----
_ISEGS = [  # name -> (bytes)
    ("xin", XR * SH * 2),        # f16 [1664, 512]
    ("wq8", D * 256),            # i8  [1536, 256]
    ("wk8", D * 256),            # i8  [1536, 256]
    ("wv", D * 256 * 2),         # f16 [1536, 256]
    ("wo", 128 * 3072 * 2),      # f16 [128, 3072]
    ("wqs", 512 * 2),            # f16 [1, 512]
    ("bqk", 128 * 4 * 4),        # f32 [128, 4]
    ("bv1", 256 * 4),            # f32 [1, 256]
    ("ones2", 128 * 2 * 4),      # f32 [128, 2]
]
IOFF = {}
_o = 0
for _n, _s in _ISEGS:
    IOFF[_n] = _o
    _o += _s
NB = _o                          # 4,068,352 bytes per core

OB_OUT8 = 0                      # i8  [1536, 512]
OB_OSCL = D * SH                 # f32 [128, 12]
OB_VST = OB_OSCL + 128 * NKT * 4  # f32 [3, 256]
OB = OB_VST + 3 * 256 * 4        # 795,648 bytes per core

# phase-1 chunks: one 512-wide chunk per AG block; block 7 only has 128
# valid columns (3584..3712) within the LP compute range.
CHUNKS = [(512 * ci, 512) for ci in range(7)] + [(3584, 128)]


def _mk_chunks(w):
    ch = [(i * w, w) for i in range(LP // w)] + [(LP - LP % w, LP % w)]
    return [(c, x) for (c, x) in ch if x > 0]

CHUNKS2 = _mk_chunks(CW2)

# core -> (F head, H head); H heads are computed on two cores each
CORE_HEADS = []
for _a in range(4):
    CORE_HEADS.append((3 * _a, 3 * _a + 1))
    CORE_HEADS.append((3 * _a + 2, 3 * _a + 1))


def _groups():
    """Dense attention groups: q ranges, kv 128-tile indices, runt kv info."""
    gs = [dict(q=[(0, 512)], kvt=list(range(4)), runt=None)]
    for b in range(NIB):
        be = IB0 + (b + 1) * 2 * FS
        kv0 = max(IB0, be - 4 * FS)
        if kv0 == IB0:
            tiles = list(range(be // 128))
        else:
            tiles = list(range(4)) + list(range(kv0 // 128, be // 128))
        q = [(IB0 + b * 2 * FS, 512), (IB0 + b * 2 * FS + 512, 512),
             (A0 + b * NAPB, NAPB)]
        gs.append(dict(q=q, kvt=tiles, runt=b))
    return gs

GROUPS = _groups()

_PROGRAM_CACHE = {}


def _install_jit_cache():
    """Cache the jitted shard_map callable across launches.

    bass2jax.run_bass_via_pjrt builds a fresh ``_body`` closure per call, so
    jax's C++ fast-path cache misses and every launch pays ~0.3s of retrace +
    executable-cache lookup (hashing the multi-MB BIR backend config).
    Reusing one jitted callable per (nc, n_cores) removes that; the per-call
    transfers and device execution are unchanged. Falls back to the original
    implementation for any case this specialized path doesn't cover.
    """
    if "patched" in _PROGRAM_CACHE:
        return
    _PROGRAM_CACHE["patched"] = True
    import jax
    from jax.sharding import Mesh, PartitionSpec
    try:
        from jax.shard_map import shard_map
    except Exception:
        from jax.experimental.shard_map import shard_map
    from concourse import bass2jax, mybir

    orig = bass2jax.run_bass_via_pjrt
    cache = {}

    def cached_run(nc, in_maps, n_cores):
        try:
            key = (id(nc), n_cores)
            ent = cache.get(key)
            if ent is None:
                if nc.dbg_addr is not None or n_cores <= 1:
                    raise RuntimeError("unsupported; use original")
                bass2jax.install_neuronx_cc_hook()
                pname = (nc.partition_id_tensor.name
                         if nc.partition_id_tensor else None)
                in_names, out_names, out_avals = [], [], []
                for alloc in nc.m.functions[0].allocations:
                    if not isinstance(alloc, mybir.MemoryLocationSet):
                        continue
                    name = alloc.memorylocations[0].name
                    if alloc.kind == "ExternalInput":
                        if name != pname:
                            in_names.append(name)
                    elif alloc.kind == "ExternalOutput":
                        out_names.append(name)
                        out_avals.append(jax.core.ShapedArray(
                            tuple(alloc.tensor_shape),
                            mybir.dt.np(alloc.dtype)))
                n_params = len(in_names)
                n_outs = len(out_avals)
                all_names = in_names + out_names + ([pname] if pname else [])
                donate = tuple(range(n_params, n_params + n_outs))

                def _body(*args):
                    operands = list(args)
                    if pname is not None:
                        operands.append(bass2jax.partition_id_tensor())
                    return tuple(bass2jax._bass_exec_p.bind(
                        *operands, out_avals=tuple(out_avals),
                        in_names=tuple(all_names),
                        out_names=tuple(out_names),
                        lowering_input_output_aliases=(),
                        sim_require_finite=True, sim_require_nnan=True,
                        nc=nc))

                devices = jax.devices()[:n_cores]
                assert len(devices) == n_cores
                mesh = Mesh(np.asarray(devices), ("core",))
                sharded = jax.jit(
                    shard_map(_body, mesh=mesh,
                              in_specs=(PartitionSpec("core"),) * (n_params + n_outs),
                              out_specs=(PartitionSpec("core"),) * n_outs,
                              check_rep=False),
                    donate_argnums=donate, keep_unused=True)
                ent = cache[key] = (sharded, in_names, out_names, out_avals)
            if ent == "disabled":
                return orig(nc, in_maps, n_cores=n_cores)
            sharded, in_names, out_names, out_avals = ent
            per_core = [[np.asarray(m[n]) for n in in_names] for m in in_maps]
            concat_in = [
                np.concatenate([per_core[c][i] for c in range(n_cores)], axis=0)
                for i in range(len(in_names))]
            concat_zeros = [
                np.zeros((n_cores * a.shape[0], *a.shape[1:]), a.dtype)
                for a in out_avals]
            try:
                out_arrs = sharded(*concat_in, *concat_zeros)
            except Exception:
                cache[key] = "disabled"
                raise
            return [
                {name: np.asarray(out_arrs[i]).reshape(
                    n_cores, *out_avals[i].shape)[c]
                 for i, name in enumerate(out_names)}
                for c in range(n_cores)]
        except Exception:
            return orig(nc, in_maps, n_cores=n_cores)

    bass2jax.run_bass_via_pjrt = cached_run


def _build_program():
    import concourse.bacc as bacc
    import concourse.tile as tile
    from concourse import mybir

    F16 = mybir.dt.float16
    F32 = mybir.dt.float32
    F32R = mybir.dt.float32r
    I8 = mybir.dt.int8
    AF = mybir.ActivationFunctionType

    nc = bacc.Bacc("TRN2", target_bir_lowering=False, debug=False, num_devices=8)

    U8 = mybir.dt.uint8

    # single packed wire input / output (one array each way: the tunnel
    # charges a fixed ~67ms round-trip per array regardless of size)
    blob = nc.dram_tensor("blob", [1, NB], U8, kind="ExternalInput")
    oblob = nc.dram_tensor("oblob", [1, OB], U8, kind="ExternalOutput")

    def iseg(name, nbytes, dt_, rearr=None, **kw):
        ap = blob.ap()[0:1, IOFF[name]:IOFF[name] + nbytes].bitcast(dt_)
        return ap.rearrange(rearr, **kw) if rearr else ap

    xin_v = iseg("xin", XR * SH * 2, F16, "a (r c) -> (a r) c", r=XR)
    wq8_v = iseg("wq8", D * 256, I8, "a (kt p c) -> (a p) kt c", kt=NKT, p=128)
    wk8_v = iseg("wk8", D * 256, I8, "a (kt p c) -> (a p) kt c", kt=NKT, p=128)
    wv_v = iseg("wv", D * 256 * 2, F16, "a (kt p c) -> (a p) kt c", kt=NKT, p=128)
    wo_v = iseg("wo", 128 * 3072 * 2, F16, "a (p c) -> (a p) c", p=128)
    wqs_v = iseg("wqs", 512 * 2, F16)
    bqk_v = iseg("bqk", 128 * 4 * 4, F32, "a (p c) -> (a p) c", p=128)
    bv1_v = iseg("bv1", 256 * 4, F32)
    ones2_v = iseg("ones2", 128 * 2 * 4, F32R, "a (p c) -> (a p) c", p=128)

    def oseg(off, nbytes, dt_, p):
        return oblob.ap()[0:1, off:off + nbytes].bitcast(dt_).rearrange(
            "a (p c) -> (a p) c", p=p)

    RG = [list(range(8))]

    with tile.TileContext(nc) as tc:
        with tc.tile_pool(name="persist", bufs=1) as P, \
             tc.tile_pool(name="xin_p", bufs=1) as XP, \
             tc.tile_pool(name="tmp", bufs=2) as T, \
             tc.tile_pool(name="pt", bufs=3) as PT, \
             tc.tile_pool(name="osb", bufs=2) as OSB, \
             tc.tile_pool(name="ps", bufs=2, space="PSUM") as PSY, \
             tc.tile_pool(name="dram", bufs=1, space="DRAM") as DR:

            # ---------- collective buffers ----------
            agin = DR.tile([XR, SH], F16, name="agin")
            agout = DR.tile([8 * XR, SH], F16, addr_space="Shared", name="agout")
            opart = DR.tile([8 * D, SH], F16, name="opart")
            rsout = DR.tile([D, SH], F16, name="rsout")
            cin = DR.tile([1, 2 * LP], F32, name="cin")
            cout = DR.tile([1, 2 * LP], F32, name="cout")

            # stage the wire shard into an internal tile, then AllGather
            nc.sync.dma_start(agin[:], xin_v)
            nc.gpsimd.collective_compute(
                "AllGather", mybir.AluOpType.bypass,
                replica_groups=RG, ins=[agin.opt()], outs=[agout.opt()])

            # ---------- phase-1-resident SBUF ----------
            wq16 = P.tile([128, NKT, 256], F16, tag="wq16", name="wq16")
            wk16 = P.tile([128, NKT, 256], F16, tag="wk16", name="wk16")
            wv16 = P.tile([128, NKT, 256], F16, tag="wv16", name="wv16")
            bqk_sb = P.tile([128, 4], F32, tag="bqk")
            bv1_sb = P.tile([1, 256], F32, tag="bv1")
            bv_sb = P.tile([128, 256], F32, tag="bv")
            ones2 = P.tile([128, 2], F32R, tag="ones2")
            zero16 = P.tile([128, SH - 96], F16, tag="zero16")
            # whole-kernel-resident
            y_q = [P.tile([128, LP], F32R, tag=f"yq{u}", name=f"yq{u}") for u in range(2)]
            y_k = [P.tile([128, LP], F32R, tag=f"yk{u}", name=f"yk{u}") for u in range(2)]
            v_sb = P.tile([128, NLT, 256], F32R, tag="vsb")

            # Wq/Wk arrive int8 with per-column fp16 scales; dequant to fp16
            # (error from q/k quantization is softmax-damped; Wv/Wo stay fp16)
            w8q_sb = P.tile([128, NKT, 256], I8, tag="w8q")
            w8k_sb = P.tile([128, NKT, 256], I8, tag="w8k")
            qks_sb = P.tile([1, 512], F16, tag="qks")
            qksb = P.tile([128, 512], F16, tag="qksb")
            nc.sync.dma_start(w8q_sb[:], wq8_v)
            nc.sync.dma_start(w8k_sb[:], wk8_v)
            nc.sync.dma_start(qks_sb[:], wqs_v)
            nc.gpsimd.partition_broadcast(qksb[:], qks_sb[:])
            for kt in range(NKT):
                for w8s, w16s, s0 in ((w8q_sb, wq16, 0), (w8k_sb, wk16, 256)):
                    wdq = T.tile([128, 256], F16, tag="wdq")
                    nc.vector.tensor_copy(wdq[:], w8s[:, kt, :])
                    nc.vector.tensor_mul(w16s[:, kt, :], wdq[:],
                                         qksb[:, s0:s0 + 256])
            nc.sync.dma_start(wv16[:], wv_v)
            nc.sync.dma_start(bqk_sb[:], bqk_v)
            nc.sync.dma_start(bv1_sb[:], bv1_v)
            nc.gpsimd.partition_broadcast(bv_sb[:], bv1_sb[:])
            nc.sync.dma_start(ones2[:], ones2_v)
            nc.vector.memset(zero16[:], 0.0)

            # zero the ReduceScatter input columns phase 3 never writes
            # (q in [3680, 4096) of block 7: states + wire padding)
            for m in range(NKT):
                r0 = 7 * D + m * 128
                nc.sync.dma_start(opart[r0:r0 + 128, 96:SH], zero16[:])

            # ---------- phase 1: projections + ssq partials ----------
            for ci, (c0, cw) in enumerate(CHUNKS):
                xr0 = XR * ci
                xc16 = XP.tile([128, NKT, SH], F16, tag="xc16")
                nc.sync.dma_start(
                    xc16[:, :, 0:cw],
                    agout[xr0:xr0 + D, 0:cw].rearrange("(kt p) l -> p kt l", p=128))
                for ti, (w16, ys) in enumerate([(wq16, y_q), (wk16, y_k)]):
                    ssq_ps = PSY.tile([1, 512], F32, tag="ssqps")
                    for u in range(2):
                        yp = PSY.tile([128, 512], F32, tag="yp")
                        for kt in range(NKT):
                            nc.tensor.matmul(
                                yp[:, 0:cw], w16[:, kt, u * 128:(u + 1) * 128],
                                xc16[:, kt, 0:cw],
                                start=(kt == 0), stop=(kt == NKT - 1))
                        nc.vector.tensor_scalar_add(
                            ys[u][:, c0:c0 + cw], yp[:, 0:cw],
                            bqk_sb[:, 2 * ti + u:2 * ti + u + 1])
                        y2 = T.tile([128, 512], F32R, tag="y2")
                        nc.scalar.activation(y2[:, 0:cw],
                                             ys[u][:, c0:c0 + cw].bitcast(F32),
                                             AF.Square)
                        nc.tensor.matmul(ssq_ps[:, 0:cw], ones2[:, u:u + 1],
                                         y2[:, 0:cw], start=(u == 0), stop=(u == 1),
                                         skip_group_check=True)
                    ssq_st = T.tile([1, 512], F32, tag="ssqst")
                    nc.vector.tensor_copy(ssq_st[:, 0:cw], ssq_ps[:, 0:cw])
                    nc.sync.dma_start(cin[0:1, ti * LP + c0:ti * LP + c0 + cw], ssq_st[:, 0:cw])
                for lt in range(c0 // 128, (c0 + cw) // 128):
                    vp = PSY.tile([128, 512], F32, tag="vp", name="vp")[:, 0:256]
                    loff = lt * 128 - c0
                    for kt in range(NKT):
                        nc.tensor.matmul(vp[:], xc16[:, kt, loff:loff + 128],
                                         wv16[:, kt, :],
                                         start=(kt == 0), stop=(kt == NKT - 1))
                    nc.vector.tensor_add(v_sb[:, lt, :], vp[:], bv_sb[:])

            nc.sync.dma_start(oseg(OB_VST, 3 * 256 * 4, F32, 3),
                              v_sb[96:99, 28, :].bitcast(F32))

            # ---------- collective: AllReduce the ssq partials ----------
            nc.gpsimd.collective_compute(
                "AllReduce", mybir.AluOpType.add,
                replica_groups=RG, ins=[cin.opt()], outs=[cout.opt()])
            eps_t = P.tile([1, 1], F32, tag="epst")
            nc.vector.memset(eps_t[:], float(EPS))

            # cos/sin fp16 shards ride in the AllGather buffer; expand the
            # duplicated halves and upconvert into SBUF (reuse weight slots)
            cos_sb = P.tile([128, LP], F32, tag="wq16", name="cos_sb")
            sin_sb = P.tile([128, LP], F32, tag="wk16", name="sin_sb")
            for ci, (c0, cw) in enumerate(CHUNKS):
                tr0 = XR * ci + D
                cs16 = T.tile([128, SH], F16, tag="cs16")
                nc.sync.dma_start(cs16[0:64, 0:cw], agout[tr0:tr0 + 64, 0:cw])
                nc.sync.dma_start(cs16[64:128, 0:cw], agout[tr0:tr0 + 64, 0:cw])
                nc.vector.tensor_copy(cos_sb[:, c0:c0 + cw], cs16[:, 0:cw])
                sn16 = T.tile([128, SH], F16, tag="sn16")
                nc.sync.dma_start(sn16[0:64, 0:cw], agout[tr0 + 64:tr0 + 128, 0:cw])
                nc.sync.dma_start(sn16[64:128, 0:cw], agout[tr0 + 64:tr0 + 128, 0:cw])
                nc.vector.tensor_copy(sin_sb[:, c0:c0 + cw], sn16[:, 0:cw])

            # ---------- phase 2: normalize + rope (in place on y) ----------
            for (c0, cw) in CHUNKS2:
                for ti, ys in enumerate([y_q, y_k]):
                    s1 = T.tile([1, CW2], F32, tag="s1")
                    nc.sync.dma_start(s1[:, 0:cw],
                                      cout[0:1, ti * LP + c0:ti * LP + c0 + cw])
                    nc.scalar.activation(s1[:, 0:cw], s1[:, 0:cw], AF.Sqrt,
                                         bias=eps_t[:, 0:1], scale=float(1.0 / D))
                    nc.vector.reciprocal(s1[:, 0:cw], s1[:, 0:cw])
                    fb = T.tile([128, CW2], F32, tag="fb")
                    nc.gpsimd.partition_broadcast(fb[:, 0:cw], s1[:, 0:cw])
                    for u in range(2):
                        y = ys[u]
                        nc.vector.tensor_mul(y[:, c0:c0 + cw],
                                             y[:, c0:c0 + cw].bitcast(F32),
                                             fb[:, 0:cw])
                        ta = T.tile([128, CW2], F32, tag="ropea")
                        tb = T.tile([128, CW2], F32, tag="ropeb")
                        tbs = T.tile([128, CW2], F32, tag="ropec")
                        yv = y[:, c0:c0 + cw].bitcast(F32)
                        nc.vector.tensor_mul(ta[:, 0:cw], yv, cos_sb[:, c0:c0 + cw])
                        nc.vector.tensor_mul(tb[:, 0:cw], yv, sin_sb[:, c0:c0 + cw])
                        nc.sync.dma_start(tbs[0:64, 0:cw], tb[64:128, 0:cw])
                        nc.sync.dma_start(tbs[64:128, 0:cw], tb[0:64, 0:cw])
                        nc.vector.tensor_sub(y[0:64, c0:c0 + cw],
                                             ta[0:64, 0:cw], tbs[0:64, 0:cw])
                        nc.vector.tensor_add(y[64:128, c0:c0 + cw],
                                             ta[64:128, 0:cw], tbs[64:128, 0:cw])

            # Wo stays fp16 (o-projection matmuls run in fp16); reuse wv slot
            wo16 = P.tile([128, 3072], F16, tag="wv16", name="wo16")
            nc.sync.dma_start(wo16[:], wo_v)

            # ---------- phase 3: attention + partial o-projection ----------
            for g in GROUPS:
                runts = []
                if g["runt"] is not None:
                    b = g["runt"]
                    a_lo = A0 + b * NAPB
                    s_row = S0 + b
                    for u in range(2):
                        kr = T.tile([128, 33], F32R, tag=f"kr{u}")
                        nc.vector.tensor_copy(kr[:, 0:32],
                                              y_k[u][:, a_lo:a_lo + 32].bitcast(F32))
                        nc.vector.tensor_copy(kr[:, 32:33],
                                              y_k[u][:, s_row:s_row + 1].bitcast(F32))
                        vr = T.tile([33, 256], F32R, tag=f"vr{u}")
                        # partition-shifting copies must go through DMA
                        nc.sync.dma_start(
                            vr[0:32, :], v_sb[32 * b:32 * b + 32, 28, :])
                        nc.sync.dma_start(
                            vr[32:33, :], v_sb[96 + b:97 + b, 28, :])
                        runts.append((kr, vr))

                kvts = g["kvt"] + ([None] if g["runt"] is not None else [])
                for (q0, qw) in g["q"]:
                    o_sb = []
                    for u in range(2):
                        oT_ps = PSY.tile([128, 512], F32, tag="vp", name="oT_ps")
                        sm_ps = PSY.tile([1, 512], F32, tag="ssqps", name="sm_ps")
                        for i, t in enumerate(kvts):
                            if t is None:
                                klhs = runts[u][0][:, :]
                                vlhs = runts[u][1][:, u * 128:(u + 1) * 128]
                                kvn = 33
                            else:
                                klhs = y_k[u][:, t * 128:(t + 1) * 128]
                                vlhs = v_sb[:, t, u * 128:(u + 1) * 128]
                                kvn = 128
                            s_ps = PSY.tile([128, 512], F32, tag="yp", name="s_ps")
                            nc.tensor.matmul(s_ps[0:kvn, 0:qw], klhs,
                                             y_q[u][:, q0:q0 + qw],
                                             start=True, stop=True)
                            pT = PT.tile([128, 512], F32R, tag="pT")
                            nc.scalar.activation(pT[0:kvn, 0:qw],
                                                 s_ps[0:kvn, 0:qw], AF.Exp,
                                                 scale=SCALE)
                            nc.tensor.matmul(oT_ps[:, 0:qw], vlhs, pT[0:kvn, 0:qw],
                                             start=(i == 0), stop=(i == len(kvts) - 1),
                                             skip_group_check=True)
                            nc.tensor.matmul(sm_ps[:, 0:qw], ones2[0:kvn, 0:1],
                                             pT[0:kvn, 0:qw],
                                             start=(i == 0), stop=(i == len(kvts) - 1),
                                             skip_group_check=True)
                        sm_sb = T.tile([1, 512], F32, tag="smsb")
                        nc.vector.reciprocal(sm_sb[:, 0:qw], sm_ps[:, 0:qw])
                        rb = T.tile([128, 512], F32, tag="rb")
                        nc.gpsimd.partition_broadcast(rb[:, 0:qw], sm_sb[:, 0:qw])
                        ot = OSB.tile([128, 512], F16, tag="ot")
                        nc.vector.tensor_mul(ot[:, 0:qw], oT_ps[:, 0:qw], rb[:, 0:qw])
                        o_sb.append(ot)
                    blk = q0 // SH
                    l0 = q0 - blk * SH
                    for m in range(NKT):
                        op_ps = PSY.tile([128, 512], F32, tag="op", name="op_ps")
                        for u in range(2):
                            nc.tensor.matmul(
                                op_ps[:, 0:qw],
                                wo16[:, u * D + m * 128:u * D + (m + 1) * 128],
                                o_sb[u][:, 0:qw],
                                start=(u == 0), stop=(u == 1))
                        op16 = OSB.tile([128, 512], F16, tag="opsb", name="op16")
                        nc.vector.tensor_copy(op16[:, 0:qw], op_ps[:, 0:qw])
                        r0 = blk * D + m * 128
                        nc.sync.dma_start(opart[r0:r0 + 128, l0:l0 + qw],
                                          op16[:, 0:qw])

            # ---------- collective: ReduceScatter the output partials ----------
            nc.gpsimd.collective_compute(
                "ReduceScatter", mybir.AluOpType.add,
                replica_groups=RG, ins=[opart.opt()], outs=[rsout.opt()])

            # quantize this core's output shard to int8 with per-feature-row
            # scales (wire compression for the D2H leg)
            ro16 = XP.tile([128, NKT, SH], F16, tag="xc16", name="ro16")
            nc.sync.dma_start(
                ro16[:], rsout[:].rearrange("(t p) l -> p t l", p=128))
            # oscl exports the exact multiplier used (host divides by it), so
            # the only round-trip error is the int8 rounding itself
            oscl_sb = P.tile([128, NKT], F32, tag="osclsb")
            for t in range(NKT):
                mx = T.tile([128, 1], F32, tag="mx")
                nc.vector.tensor_reduce(mx[:], ro16[:, t, :],
                                        axis=mybir.AxisListType.X,
                                        op=mybir.AluOpType.max,
                                        apply_absolute_value=True)
                nc.vector.tensor_scalar_max(mx[:], mx[:], 1e-2)
                rr = T.tile([128, 1], F32, tag="rr")
                nc.vector.reciprocal(rr[:], mx[:])
                nc.vector.tensor_scalar_mul(rr[:], rr[:], 127.0)
                nc.vector.tensor_copy(oscl_sb[:, t:t + 1], rr[:])
                q8t = OSB.tile([128, SH], I8, tag="q8t")
                nc.vector.tensor_scalar_mul(q8t[:], ro16[:, t, :], rr[:, 0:1])
                nc.sync.dma_start(oseg(128 * SH * t, 128 * SH, I8, 128), q8t[:])
            nc.sync.dma_start(oseg(OB_OSCL, 128 * NKT * 4, F32, 128),
                              oscl_sb[:])

    nc.finalize()
    return nc


def _prep_inputs(x, freqs, freqs_action, freqs_state, Wq, bq, Wk, bk, Wv, bv,
                 Wo, bo, gq, gk):
    """Host-side input prep -> per-core in_maps. gq/gk are ones (per spec)."""
    x = np.ascontiguousarray(np.asarray(x, np.float32)[0])
    xT16 = np.zeros((D, LPAD), np.float16)
    xT16[:, :L] = x.T.astype(np.float16)
    f = np.concatenate([np.asarray(freqs), np.asarray(freqs_action),
                        np.asarray(freqs_state)], 0).astype(np.float32)
    f = f.reshape(L, HD // 2, 2)
    cos64 = np.zeros((64, LPAD), np.float16)
    sin64 = np.zeros((64, LPAD), np.float16)
    cos64[:, :L] = f[..., 0].T.astype(np.float16)
    sin64[:, :L] = f[..., 1].T.astype(np.float16)
    perm = np.concatenate([np.arange(0, HD, 2), np.arange(1, HD, 2)])
    ones2 = np.ones((128, 2), np.float32)
    ones2[:, 1] = 0.5

    Wq = np.asarray(Wq, np.float32); Wk = np.asarray(Wk, np.float32)
    Wv = np.asarray(Wv, np.float32); Wo = np.asarray(Wo, np.float32)
    bq = np.asarray(bq, np.float32); bk = np.asarray(bk, np.float32)
    bv = np.asarray(bv, np.float32)

    def quant8(w):
        # per-column symmetric int8; scale kept in fp16 (as the device uses it)
        s = (np.abs(w).max(0) / 127.0).astype(np.float16)
        s32 = np.maximum(s.astype(np.float32), 1e-12)
        q = np.clip(np.round(w / s32[None, :]), -127, 127).astype(np.int8)
        return q, s

    in_maps = []
    for c in range(8):
        F, H = CORE_HEADS[c]
        pf = F * HD + perm
        ph = H * HD + perm
        vcols = np.r_[F * HD:(F + 1) * HD, H * HD:(H + 1) * HD]
        sl = slice(SH * c, SH * (c + 1))
        q8, qs = quant8(np.concatenate([Wq[:, pf], Wq[:, ph]], 1))
        k8, ks = quant8(np.concatenate([Wk[:, pf], Wk[:, ph]], 1))
        segs = {
            "xin": np.ascontiguousarray(np.concatenate(
                [xT16[:, sl], cos64[:, sl], sin64[:, sl]], 0)),
            "wq8": np.ascontiguousarray(q8),
            "wk8": np.ascontiguousarray(k8),
            "wv": np.ascontiguousarray(Wv[:, vcols]).astype(np.float16),
            "wo": np.ascontiguousarray(np.concatenate(
                [Wo[F * HD:(F + 1) * HD, :], 0.5 * Wo[H * HD:(H + 1) * HD, :]],
                1)).astype(np.float16),
            "wqs": np.ascontiguousarray(np.concatenate([qs, ks])[None, :]),
            "bqk": np.ascontiguousarray(
                np.stack([bq[pf], bq[ph], bk[pf], bk[ph]], 1).astype(np.float32)),
            "bv1": np.ascontiguousarray(bv[vcols][None, :].astype(np.float32)),
            "ones2": ones2,
        }
        blob = np.concatenate(
            [np.ascontiguousarray(segs[n]).view(np.uint8).reshape(1, -1)
             for n, _ in _ISEGS], axis=1)
        assert blob.shape == (1, NB)
        in_maps.append({"blob": blob})
    return in_maps


def kernel(**inputs) -> np.ndarray:
    from concourse.bass_utils import run_bass_kernel_spmd

    _install_jit_cache()
    if "nc" not in _PROGRAM_CACHE:
        _PROGRAM_CACHE["nc"] = _build_program()
    nc = _PROGRAM_CACHE["nc"]

    in_maps = _prep_inputs(**inputs)
    res = run_bass_kernel_spmd(nc, in_maps, core_ids=list(range(8)))

    Wo = np.asarray(inputs["Wo"], np.float32)
    bo = np.asarray(inputs["bo"], np.float32)
    outT = np.zeros((D, LPAD), np.float32)
    vsts = []
    for c in range(8):
        ob = res.results[c]["oblob"][0]
        q8 = ob[:OB_OSCL].view(np.int8).reshape(D, SH).astype(np.float32)
        rr = ob[OB_OSCL:OB_VST].view(np.float32).reshape(128, NKT)
        vsts.append(ob[OB_VST:].view(np.float32).reshape(3, 256))
        s = np.ascontiguousarray(rr.T).reshape(D, 1)   # feature d = 128*t + p
        outT[:, SH * c:SH * (c + 1)] = q8 / s
    out = np.zeros((L, D), np.float32)
    out[:S0] = outT[:, :S0].T
    v_state = np.zeros((3, D), np.float32)
    have = set()
    for c in range(8):
        F, H = CORE_HEADS[c]
        vs = vsts[c]
        if F not in have:
            v_state[:, F * HD:(F + 1) * HD] = vs[:, :HD]
            have.add(F)
        if H not in have:
            v_state[:, H * HD:(H + 1) * HD] = vs[:, HD:]
            have.add(H)
    out[S0:S0 + NIB] = v_state @ Wo
    out += bo[None, :]
    return out[None].astype(np.float32)
